# revision 1
# baseline (speedup 1.0000x reference)
"""GQA attention kernel for Trainium2, 8-core SPMD.

Sharding: core c = 2*b + g handles batch b (of 4) and head-group g (of 2):
8 of 16 q-heads, 2 of 4 kv-heads.  Each core computes its partial
out^T = (attn_out @ wo_g^T)^T in transposed space (no on-chip transposes);
the host adds the two group partials per batch and transposes back.

Everything on-chip is computed in transposed orientation:
  Q^T/K^T: [head_dim(part), T]   scores^T: [kt(part), qt]   O^T: [d(part), qt]
RoPE is handled by permuting wq/wk rows on the host to an
[evens | odds] layout (scores are invariant to a shared d-permutation).
Matmuls run as float32r (fp32 storage, full-rate PE path).
Softmax skips the max-subtraction (scores are O(1) by construction) and the
causal mask is applied by zeroing exp(S^T) tiles with gpsimd.affine_select.
"""

import math
import numpy as np

B, T, C = 4, 2048, 2048
N_HEAD, N_KV_HEAD, HD = 16, 4, 128
N_CORES = 8
SCALE = 1.0 / math.sqrt(HD)

_PROG = {}
_LAST_IN_MAPS = None


def _build_program():
    from contextlib import ExitStack
    import concourse.bacc as bacc
    import concourse.mybir as mybir
    import concourse.tile as tile

    f32 = mybir.dt.float32
    f32r = mybir.dt.float32r
    Exp = mybir.ActivationFunctionType.Exp

    nc = bacc.Bacc(None, target_bir_lowering=False)
    xT = nc.declare_dram_parameter("xT", [C, T], f32, isOutput=False)
    wqT = nc.declare_dram_parameter("wqT", [C, 1024], f32, isOutput=False)
    wkT = nc.declare_dram_parameter("wkT", [C, 256], f32, isOutput=False)
    wvT = nc.declare_dram_parameter("wvT", [C, 256], f32, isOutput=False)
    woT = nc.declare_dram_parameter("woT", [1024, T], f32, isOutput=False)
    cosT = nc.declare_dram_parameter("cosT", [64, T], f32, isOutput=False)
    pswapD = nc.declare_dram_parameter("pswap", [128, 128], f32, isOutput=False)
    sinT = nc.declare_dram_parameter("sinT", [64, T], f32, isOutput=False)
    out = nc.declare_dram_parameter("out", [C, T], f32, isOutput=True)
    Qd = nc.dram_tensor("Qd", [8, 128, T], f32)
    Od = nc.dram_tensor("Od", [8, 128, T], f32)

    with tile.TileContext(nc) as tc, nc.allow_low_precision(
        reason="float32r tiles hold full fp32 bits"
    ), ExitStack() as top:
        consts = top.enter_context(tc.tile_pool(name="consts", bufs=1))
        # cs2 = [cos; cos] stacked to 128 partitions; sb2 = [-sin; +sin] so
        # rope(x) = x * cs2 + swap_halves(x) * sb2 with full-width DVE ops
        cs2 = consts.tile([128, T], f32)
        sb2 = consts.tile([128, T], f32)
        nc.sync.dma_start(out=cs2[0:64, :], in_=cosT[:])
        nc.sync.dma_start(out=cs2[64:128, :], in_=cosT[:])
        nc.sync.dma_start(out=sb2[0:64, :], in_=sinT[:])
        nc.sync.dma_start(out=sb2[64:128, :], in_=sinT[:])
        nc.vector.tensor_scalar_mul(sb2[0:64, :], sb2[0:64, :], -1.0)
        pswap = consts.tile([128, 128], f32r)
        nc.sync.dma_start(out=pswap, in_=pswapD[:].bitcast(f32r))
        ones_f = consts.tile([128, 1], f32)
        ones_rf = consts.tile([1, 128], f32)
        nc.vector.memset(ones_f, 1.0)
        nc.vector.memset(ones_rf, 1.0)
        ones_col = consts.tile([128, 1], f32r)
        ones_row = consts.tile([1, 128], f32r)
        nc.vector.tensor_copy(ones_col, ones_f)
        nc.vector.tensor_copy(ones_row, ones_rf)
        K_sb = consts.tile([128, 2, T], f32r)   # rotated K^T per kv head
        V_sb = consts.tile([128, 16, 256], f32r)  # V[t(part), ti, kv*128+d]

        # ---- phase 1: QKV projections (two passes over xT), K RoPE ----
        with ExitStack() as ph1:
            wpool = ph1.enter_context(tc.tile_pool(name="wpool", bufs=1))
            wq_sb = wpool.tile([128, 16, 1024], f32r)
            wk_sb = wpool.tile([128, 16, 256], f32r)
            wv_sb = wpool.tile([128, 16, 256], f32r)
            nc.sync.dma_start(
                out=wq_sb, in_=wqT.rearrange("(n p) m -> p n m", p=128).bitcast(f32r)
            )
            nc.sync.dma_start(
                out=wk_sb, in_=wkT.rearrange("(n p) m -> p n m", p=128).bitcast(f32r)
            )
            nc.sync.dma_start(
                out=wv_sb, in_=wvT.rearrange("(n p) m -> p n m", p=128).bitcast(f32r)
            )
            xs = ph1.enter_context(tc.tile_pool(name="xs", bufs=8))
            stage = ph1.enter_context(tc.tile_pool(name="stage", bufs=6))
            raws = ph1.enter_context(tc.tile_pool(name="raws", bufs=2))
            ropes = ph1.enter_context(tc.tile_pool(name="ropes", bufs=2))
            # pass A: Q projection -> Qd (raw, RoPE applied at load in phase 2)
            with ExitStack() as pa:
                pqA = pa.enter_context(tc.tile_pool(name="pqA", bufs=8, space="PSUM"))
                for t4 in range(4):
                    tsl = slice(t4 * 512, (t4 + 1) * 512)
                    q_ps = [pqA.tile([128, 512], f32, tag="qps", name=f"qps{i}")
                            for i in range(8)]
                    for ci in range(16):
                        xt = xs.tile([128, 512], f32r, name="xt")
                        nc.sync.dma_start(
                            out=xt, in_=xT[ci * 128:(ci + 1) * 128, tsl].bitcast(f32r)
                        )
                        for h in range(8):
                            nc.tensor.matmul(
                                q_ps[h], wq_sb[:, ci, h * 128:(h + 1) * 128], xt,
                                start=(ci == 0), stop=(ci == 15),
                            )
                    for h in range(8):
                        qst = stage.tile([128, 512], f32, tag="qst", name="qst")
                        nc.scalar.copy(qst, q_ps[h])
                        nc.sync.dma_start(out=Qd[h, :, tsl], in_=qst)
            # pass B: K/V projections + K RoPE
            with ExitStack() as pb:
                pkB = pb.enter_context(tc.tile_pool(name="pkB", bufs=2, space="PSUM"))
                pvB = pb.enter_context(tc.tile_pool(name="pvB", bufs=4, space="PSUM"))
                pswp = pb.enter_context(tc.tile_pool(name="pswp", bufs=2, space="PSUM"))
                for t4 in range(4):
                    tsl = slice(t4 * 512, (t4 + 1) * 512)
                    k_ps = [pkB.tile([128, 512], f32, tag="kps", name=f"kps{i}")
                            for i in range(2)]
                    v_ps = [pvB.tile([128, 256], f32, tag="vps", name=f"vps{i}")
                            for i in range(4)]
                    for ci in range(16):
                        xt = xs.tile([128, 512], f32r, name="xt")
                        nc.sync.dma_start(
                            out=xt, in_=xT[ci * 128:(ci + 1) * 128, tsl].bitcast(f32r)
                        )
                        for kv in range(2):
                            nc.tensor.matmul(
                                k_ps[kv], wk_sb[:, ci, kv * 128:(kv + 1) * 128], xt,
                                start=(ci == 0), stop=(ci == 15),
                            )
                        for sub in range(4):
                            nc.tensor.matmul(
                                v_ps[sub], xt[:, sub * 128:(sub + 1) * 128],
                                wv_sb[:, ci, :],
                                start=(ci == 0), stop=(ci == 15),
                            )
                    for sub in range(4):
                        nc.scalar.copy(V_sb[:, t4 * 4 + sub, :], v_ps[sub])
                    for kv in range(2):
                        raw = raws.tile([128, 512], f32r, tag="raw", name="raw")
                        nc.scalar.copy(raw, k_ps[kv])
                        swp_ps = pswp.tile([128, 512], f32, tag="swpps", name="swp_ps")
                        nc.tensor.matmul(swp_ps, pswap, raw)
                        ta = ropes.tile([128, 512], f32, tag="ta", name="ta")
                        tb = ropes.tile([128, 512], f32, tag="tb", name="tb")
                        nc.vector.tensor_mul(ta, raw, cs2[:, tsl])
                        nc.vector.tensor_mul(tb, swp_ps, sb2[:, tsl])
                        nc.vector.tensor_add(K_sb[:, kv, tsl], ta, tb)

        # ---- phase 2: causal attention (S^T orientation) ----
        with ExitStack() as ph2:
            qload = ph2.enter_context(tc.tile_pool(name="qload", bufs=4))
            pwork = ph2.enter_context(tc.tile_pool(name="pwork", bufs=6))
            dwork = ph2.enter_context(tc.tile_pool(name="dwork", bufs=2))
            small = ph2.enter_context(tc.tile_pool(name="small", bufs=2))
            rbp = ph2.enter_context(tc.tile_pool(name="rbp", bufs=2))
            osb = ph2.enter_context(tc.tile_pool(name="osb", bufs=4))
            qrope = ph2.enter_context(tc.tile_pool(name="qrope", bufs=2))
            ps_s = ph2.enter_context(tc.tile_pool(name="ps_s", bufs=3, space="PSUM"))
            pswp2 = ph2.enter_context(tc.tile_pool(name="pswp2", bufs=1, space="PSUM"))
            ps_o = ph2.enter_context(tc.tile_pool(name="ps_o", bufs=2, space="PSUM"))
            ps_m = ph2.enter_context(tc.tile_pool(name="ps_m", bufs=2, space="PSUM"))

            for h in range(8):
                kv = h // 4
                for qj in range(4):
                    qsl = slice(qj * 512, (qj + 1) * 512)
                    qraw = qload.tile([128, 512], f32r, name="qraw")
                    nc.sync.dma_start(out=qraw, in_=Qd[h, :, qsl].bitcast(f32r))
                    swp_ps = pswp2.tile([128, 512], f32, tag="swpps2", name="swp_ps2")
                    nc.tensor.matmul(swp_ps, pswap, qraw)
                    ta = qrope.tile([128, 512], f32, tag="qta", name="qta")
                    tb = qrope.tile([128, 512], f32, tag="qtb", name="qtb")
                    nc.vector.tensor_mul(ta, qraw, cs2[:, qsl])
                    nc.vector.tensor_mul(tb, swp_ps, sb2[:, qsl])
                    qt = qload.tile([128, 512], f32r, name="qt")
                    nc.vector.tensor_add(qt, ta, tb)
                    den = dwork.tile([128, 512], f32r, tag="den", name="den")
                    o_ps = ps_o.tile([128, 512], f32, name="o_ps")
                    nk = 4 * (qj + 1)
                    for ki in range(nk):
                        s_ps = ps_s.tile([128, 512], f32, name="s_ps")
                        nc.tensor.matmul(
                            s_ps, K_sb[:, kv, ki * 128:(ki + 1) * 128], qt
                        )
                        p = pwork.tile([128, 512], f32r, tag="p", name="p")
                        nc.scalar.activation(p, s_ps, Exp, scale=SCALE)
                        if ki >= 4 * qj:
                            nc.gpsimd.affine_select(
                                out=p, in_=p, pattern=[[1, 512]],
                                compare_op=mybir.AluOpType.is_ge, fill=0.0,
                                base=qj * 512 - ki * 128, channel_multiplier=-1,
                            )
                        if ki == 0:
                            nc.vector.tensor_copy(den, p)
                        else:
                            nc.vector.tensor_add(den, den, p)
                        nc.tensor.matmul(
                            o_ps, V_sb[:, ki, kv * 128:(kv + 1) * 128], p,
                            start=(ki == 0), stop=(ki == nk - 1),
                        )
                    den_ps = ps_m.tile([1, 512], f32, tag="sm", name="den_ps")
                    nc.tensor.matmul(den_ps, ones_col, den)
                    recip = small.tile([1, 512], f32r, tag="recip", name="recip")
                    nc.vector.reciprocal(recip, den_ps[0:1, :])
                    bc_ps = ps_m.tile([128, 512], f32, tag="sm", name="bc_ps")
                    nc.tensor.matmul(bc_ps, ones_row, recip[0:1, :])
                    rb = rbp.tile([128, 512], f32, tag="rb", name="rb")
                    nc.scalar.copy(rb, bc_ps)
                    o_sb = osb.tile([128, 512], f32, name="o_sb")
                    nc.vector.tensor_mul(o_sb, o_ps, rb)
                    nc.sync.dma_start(
                        out=Od[h, :, qj * 512:(qj + 1) * 512], in_=o_sb
                    )

        # ---- phase 3: output projection (transposed partials) ----
        with ExitStack() as ph3:
            wop = ph3.enter_context(tc.tile_pool(name="wop", bufs=1))
            wo_sb = wop.tile([128, 8, T], f32r)
            nc.sync.dma_start(
                out=wo_sb, in_=woT.rearrange("(h p) e -> p h e", p=128).bitcast(f32r)
            )
            oload = ph3.enter_context(tc.tile_pool(name="oload", bufs=2))
            outsb = ph3.enter_context(tc.tile_pool(name="outsb", bufs=6))
            ps_out = ph3.enter_context(tc.tile_pool(name="ps_out", bufs=6, space="PSUM"))
            for tj in range(4):
                o_sl = oload.tile([128, 8, 512], f32r, name="o_sl")
                nc.sync.dma_start(
                    out=o_sl,
                    in_=Od[:, :, tj * 512:(tj + 1) * 512]
                    .rearrange("h p t -> p h t").bitcast(f32r),
                )
                for e in range(16):
                    op_ = ps_out.tile([128, 512], f32, name="op")
                    for h in range(8):
                        nc.tensor.matmul(
                            op_, wo_sb[:, h, e * 128:(e + 1) * 128], o_sl[:, h, :],
                            start=(h == 0), stop=(h == 7),
                        )
                    ob = outsb.tile([128, 512], f32, name="ob")
                    nc.scalar.copy(ob, op_)
                    nc.sync.dma_start(
                        out=out[e * 128:(e + 1) * 128, tj * 512:(tj + 1) * 512], in_=ob
                    )

    nc.compile()
    return nc


def _get_program():
    if "nc" not in _PROG:
        _PROG["nc"] = _build_program()
    return _PROG["nc"]


def kernel(x, wq, wk, wv, wo, rope_cos, rope_sin):
    from concourse.bass_utils import run_bass_kernel_spmd

    nc = _get_program()
    x = np.asarray(x, dtype=np.float32)
    wq = np.asarray(wq, dtype=np.float32)
    wk = np.asarray(wk, dtype=np.float32)
    wv = np.asarray(wv, dtype=np.float32)
    wo = np.asarray(wo, dtype=np.float32)
    rope_cos = np.asarray(rope_cos, dtype=np.float32)
    rope_sin = np.asarray(rope_sin, dtype=np.float32)

    # even/odd -> [evens | odds] permutation of each head's rows of wq/wk
    perm = np.concatenate([np.arange(0, HD, 2), np.arange(1, HD, 2)])
    wq_p = wq.reshape(N_HEAD, HD, C)[:, perm, :]
    wk_p = wk.reshape(N_KV_HEAD, HD, C)[:, perm, :]

    pswap = np.zeros((128, 128), dtype=np.float32)
    pswap[(np.arange(128) + 64) % 128, np.arange(128)] = 1.0
    cosT = np.ascontiguousarray(rope_cos.T)
    sinT = np.ascontiguousarray(rope_sin.T)

    in_maps = []
    for core in range(N_CORES):
        b, g = core // 2, core % 2
        wq_g = wq_p[8 * g:8 * g + 8].reshape(1024, C)
        wk_g = wk_p[2 * g:2 * g + 2].reshape(256, C)
        wv_g = wv.reshape(N_KV_HEAD, HD, C)[2 * g:2 * g + 2].reshape(256, C)
        in_maps.append({
            "xT": np.ascontiguousarray(x[b].T),
            "wqT": np.ascontiguousarray(wq_g.T),
            "wkT": np.ascontiguousarray(wk_g.T),
            "wvT": np.ascontiguousarray(wv_g.T),
            "woT": np.ascontiguousarray(wo[:, 1024 * g:1024 * (g + 1)].T),
            "pswap": pswap,
            "cosT": cosT,
            "sinT": sinT,
        })

    global _LAST_IN_MAPS
    _LAST_IN_MAPS = in_maps
    res = run_bass_kernel_spmd(nc, in_maps, list(range(N_CORES))).results
    out = np.empty((B, T, C), dtype=np.float32)
    for b in range(B):
        out[b] = (res[2 * b]["out"] + res[2 * b + 1]["out"]).T
    return out



# revision 6
# speedup vs baseline: 1.5196x; 1.5196x over previous
"""GQA attention kernel for Trainium2, 8-core SPMD.

Sharding: core c = 2*b + g handles batch b (of 4) and head-group g (of 2):
8 of 16 q-heads, 2 of 4 kv-heads.  Each core computes its partial
out^T = (attn_out @ wo_g^T)^T in transposed space (no on-chip transposes);
the host adds the two group partials per batch and transposes back.

v2 design (vs the f32r baseline):
  - fp16 storage everywhere (PE runs 1 cycle/row for fp16 just like f32r,
    but DVE gets 2x and DMA/SBUF halve); PSUM accumulation stays fp32.
  - x resident in SBUF (loaded once); Q and O never leave SBUF: no DRAM
    round-trips.
  - causal mask applied by PRELOADING -30000*mask into the scores PSUM
    bank via an extra matmul (negeye @ cmask) before the K^T q matmul
    accumulates on top; exp then yields exact zeros.  No gpsimd
    affine_select in the exp->AV chain.
  - softmax denominator: DVE adds of exp tiles (fp16), partition-reduce
    via ones-column matmul -> [1,512] PSUM, reciprocal_approx_fast (DVE),
    gpsimd partition_broadcast, one DVE multiply to normalize.
  - per-head software pipeline: Q-projection (and RoPE) of head h+1 is
    emitted interleaved into attention of head h as Tensor filler work, so
    the PE array stays busy during the cross-engine softmax chain.

Everything on-chip is computed in transposed orientation:
  Q^T/K^T: [head_dim(part), T]   scores^T: [kt(part), qt]   O^T: [d(part), qt]
RoPE is handled by permuting wq/wk rows on the host to an
[evens | odds] layout (scores are invariant to a shared d-permutation).
"""

import math
import numpy as np

B, T, C = 4, 2048, 2048
N_HEAD, N_KV_HEAD, HD = 16, 4, 128
N_CORES = 8
SCALE = 1.0 / math.sqrt(HD)
NEG = -30000.0

_PROG = {}
_LAST_IN_MAPS = None


def _build_program():
    from contextlib import ExitStack
    import concourse.bacc as bacc
    import concourse.mybir as mybir
    import concourse.tile as tile

    f16 = mybir.dt.float16
    f32 = mybir.dt.float32
    Exp = mybir.ActivationFunctionType.Exp

    nc = bacc.Bacc(None, target_bir_lowering=False)
    xH = nc.declare_dram_parameter("xH", [128, 16, T], f16, isOutput=False)
    wqH = nc.declare_dram_parameter("wqH", [8, 128, 16, 128], f16, isOutput=False)
    wkH = nc.declare_dram_parameter("wkH", [128, 16, 256], f16, isOutput=False)
    wvH = nc.declare_dram_parameter("wvH", [128, 16, 256], f16, isOutput=False)
    woH = nc.declare_dram_parameter("woH", [128, 8, T], f16, isOutput=False)
    cos2H = nc.declare_dram_parameter("cos2", [128, T], f16, isOutput=False)
    sin2H = nc.declare_dram_parameter("sin2", [128, T], f32, isOutput=False)
    pswapH = nc.declare_dram_parameter("pswap", [128, 128], f16, isOutput=False)
    negeyeH = nc.declare_dram_parameter("negeye", [128, 128], f16, isOutput=False)
    cmaskH = nc.declare_dram_parameter("cmask", [128, 4, 512], f16, isOutput=False)
    out = nc.declare_dram_parameter("out", [C, T], f16, isOutput=True)

    with tile.TileContext(nc) as tc, nc.allow_low_precision(
        reason="fp16 storage with fp32 PSUM accumulation; tolerance is 2e-2"
    ), ExitStack() as top:
        consts = top.enter_context(tc.tile_pool(name="consts", bufs=1))
        cs2 = consts.tile([128, T], f16)
        sn2 = consts.tile([128, T], f32)  # f32: multiplied against PSUM f32
        pswap = consts.tile([128, 128], f16)
        negeye = consts.tile([128, 128], f16)
        cmask = consts.tile([128, 4, 512], f16)
        ones_col = consts.tile([128, 1], f16)
        nc.sync.dma_start(out=cs2, in_=cos2H[:])
        nc.sync.dma_start(out=sn2, in_=sin2H[:])
        nc.sync.dma_start(out=pswap, in_=pswapH[:])
        nc.sync.dma_start(out=negeye, in_=negeyeH[:])
        nc.sync.dma_start(out=cmask, in_=cmaskH[:])
        nc.vector.memset(ones_col, 1.0)

        data = top.enter_context(tc.tile_pool(name="data", bufs=1))
        x_sb = data.tile([128, 16, T], f16)
        K_sb = data.tile([128, 2, T], f16)
        V_sb = data.tile([128, 16, 256], f16)
        O_sb = data.tile([128, 8, T], f16)
        wo_sb = data.tile([128, 8, T], f16)
        for ci in range(16):
            nc.sync.dma_start(out=x_sb[:, ci, :], in_=xH[:, ci, :])

        # pools shared by K-rope (KV pass) and Q-rope (attention era)
        qraws = top.enter_context(tc.tile_pool(name="qraws", bufs=2))
        ropes = top.enter_context(tc.tile_pool(name="ropes", bufs=4))
        ps_swp = top.enter_context(tc.tile_pool(name="ps_swp", bufs=1, space="PSUM"))

        def emit_rope(raw_f16, swp_ps, dst):
            # dst = raw*cs2 + swap_halves(raw)*sn2, all [128, 512] slices
            tsl = dst[1]
            ta = ropes.tile([128, 512], f16, tag="ta", name="ta")
            tb = ropes.tile([128, 512], f16, tag="tb", name="tb")
            nc.vector.tensor_mul(ta, raw_f16, cs2[:, tsl])
            nc.vector.tensor_mul(tb, swp_ps, sn2[:, tsl])
            nc.vector.tensor_add(dst[0], ta, tb)

        # ---- phase 1: K/V projections + K RoPE (x resident in SBUF) ----
        with ExitStack() as kv_stack:
            wkv = kv_stack.enter_context(tc.tile_pool(name="wkv", bufs=1))
            wk_sb = wkv.tile([128, 16, 256], f16)
            wv_sb = wkv.tile([128, 16, 256], f16)
            nc.sync.dma_start(out=wk_sb, in_=wkH[:])
            nc.sync.dma_start(out=wv_sb, in_=wvH[:])
            ps_k = kv_stack.enter_context(tc.tile_pool(name="ps_k", bufs=2, space="PSUM"))
            ps_v = kv_stack.enter_context(tc.tile_pool(name="ps_v", bufs=4, space="PSUM"))
            for t4 in range(4):
                tsl = slice(t4 * 512, (t4 + 1) * 512)
                k_ps = [ps_k.tile([128, 512], f32, tag="kps", name=f"kps{i}")
                        for i in range(2)]
                v_ps = [ps_v.tile([128, 256], f32, tag="vps", name=f"vps{i}")
                        for i in range(4)]
                for ci in range(16):
                    for kv in range(2):
                        nc.tensor.matmul(
                            k_ps[kv], wk_sb[:, ci, kv * 128:(kv + 1) * 128],
                            x_sb[:, ci, tsl], start=(ci == 0), stop=(ci == 15),
                        )
                    for sub in range(4):
                        nc.tensor.matmul(
                            v_ps[sub],
                            x_sb[:, ci, t4 * 512 + sub * 128:t4 * 512 + (sub + 1) * 128],
                            wv_sb[:, ci, :], start=(ci == 0), stop=(ci == 15),
                        )
                for sub in range(4):
                    nc.scalar.copy(V_sb[:, t4 * 4 + sub, :], v_ps[sub])
                for kv in range(2):
                    raw = qraws.tile([128, 512], f16, tag="raw", name="raw")
                    nc.scalar.copy(raw, k_ps[kv])
                    swp_ps = ps_swp.tile([128, 512], f32, tag="swp", name="swp")
                    nc.tensor.matmul(swp_ps, pswap, raw)
                    emit_rope(raw, swp_ps, (K_sb[:, kv, tsl], tsl))

        # ---- attention era: per-head Qproj pipelined with attention ----
        wqp = top.enter_context(tc.tile_pool(name="wqp", bufs=2))
        qsbp = top.enter_context(tc.tile_pool(name="qsbp", bufs=2))
        pwork = top.enter_context(tc.tile_pool(name="pwork", bufs=6))
        dwork = top.enter_context(tc.tile_pool(name="dwork", bufs=2))
        small = top.enter_context(tc.tile_pool(name="small", bufs=2))
        rbcp = top.enter_context(tc.tile_pool(name="rbcp", bufs=2))
        with ExitStack() as at_stack:
            ps_qp = at_stack.enter_context(tc.tile_pool(name="ps_qp", bufs=1, space="PSUM"))
            ps_s = at_stack.enter_context(tc.tile_pool(name="ps_s", bufs=3, space="PSUM"))
            ps_o = at_stack.enter_context(tc.tile_pool(name="ps_o", bufs=2, space="PSUM"))
            ps_den = at_stack.enter_context(tc.tile_pool(name="ps_den", bufs=1, space="PSUM"))

            q_tiles = {}

            def qproj_ops(h):
                """Closure list computing Q_sb for head h (proj + rope)."""
                ops = []
                wq_sb = wqp.tile([128, 16, 128], f16, tag="wq", name=f"wq{h}")
                q_sb = qsbp.tile([128, T], f16, tag="q", name=f"q{h}")
                q_tiles[h] = q_sb
                ops.append(lambda: nc.sync.dma_start(out=wq_sb, in_=wqH[h]))
                for t4 in range(4):
                    tsl = slice(t4 * 512, (t4 + 1) * 512)
                    q_ps = ps_qp.tile([128, 512], f32, tag="qp", name="qp")
                    for ci in range(16):
                        ops.append(lambda q_ps=q_ps, ci=ci, tsl=tsl: nc.tensor.matmul(
                            q_ps, wq_sb[:, ci, :], x_sb[:, ci, tsl],
                            start=(ci == 0), stop=(ci == 15),
                        ))
                    def rope_q(q_ps=q_ps, tsl=tsl):
                        raw = qraws.tile([128, 512], f16, tag="raw", name="raw")
                        nc.scalar.copy(raw, q_ps)
                        swp_ps = ps_swp.tile([128, 512], f32, tag="swp", name="swp")
                        nc.tensor.matmul(swp_ps, pswap, raw)
                        emit_rope(raw, swp_ps, (q_sb[:, tsl], tsl))
                    ops.append(rope_q)
                return ops

            def emit_attn(h, filler):
                kv = h // 4
                q_sb = q_tiles.pop(h)
                for qj in range(4):
                    qsl = slice(qj * 512, (qj + 1) * 512)
                    nk = 4 * (qj + 1)
                    o_ps = ps_o.tile([128, 512], f32, tag="o", name="o_ps")
                    den = dwork.tile([128, 512], f16, tag="den", name="den")
                    for ki in range(nk):
                        r = ki - 4 * qj
                        s_ps = ps_s.tile([128, 512], f32, tag="s", name="s_ps")
                        if r >= 0:
                            nc.tensor.matmul(
                                s_ps, negeye, cmask[:, r, :], start=True, stop=False
                            )
                        nc.tensor.matmul(
                            s_ps, K_sb[:, kv, ki * 128:(ki + 1) * 128], q_sb[:, qsl],
                            start=(r < 0), stop=True,
                        )
                        for _ in range(2):
                            if filler:
                                filler.pop(0)()
                        p = pwork.tile([128, 512], f16, tag="p", name="p")
                        nc.scalar.activation(p, s_ps, Exp, scale=SCALE)
                        if ki == 0:
                            nc.vector.tensor_copy(den, p)
                        else:
                            nc.vector.tensor_add(den, den, p)
                        nc.tensor.matmul(
                            o_ps, V_sb[:, ki, kv * 128:(kv + 1) * 128], p,
                            start=(ki == 0), stop=(ki == nk - 1),
                        )
                        if filler:
                            filler.pop(0)()
                    den_ps = ps_den.tile([1, 512], f32, tag="dn", name="den_ps")
                    nc.tensor.matmul(den_ps, ones_col, den)
                    recip = small.tile([1, 512], f32, tag="rc", name="recip")
                    nc.vector.reciprocal_approx_fast(out=recip, in_=den_ps)
                    rbc = rbcp.tile([128, 512], f32, tag="rbc", name="rbc")
                    nc.gpsimd.partition_broadcast(rbc, recip)
                    nc.vector.tensor_mul(O_sb[:, h, qsl], o_ps, rbc)

            nc.sync.dma_start(out=wo_sb, in_=woH[:])
            for op in qproj_ops(0):
                op()
            for h in range(8):
                filler = qproj_ops(h + 1) if h < 7 else []
                emit_attn(h, filler)
                for op in filler:
                    op()

        # ---- phase 3: output projection (transposed partials) ----
        with ExitStack() as ph3:
            outsb = ph3.enter_context(tc.tile_pool(name="outsb", bufs=4))
            ps_out = ph3.enter_context(tc.tile_pool(name="ps_out", bufs=6, space="PSUM"))
            for tj in range(4):
                tsl = slice(tj * 512, (tj + 1) * 512)
                for e in range(16):
                    op_ = ps_out.tile([128, 512], f32, tag="op", name="op")
                    for hh in range(8):
                        nc.tensor.matmul(
                            op_, wo_sb[:, hh, e * 128:(e + 1) * 128],
                            O_sb[:, hh, tsl], start=(hh == 0), stop=(hh == 7),
                        )
                    ob = outsb.tile([128, 512], f16, tag="ob", name="ob")
                    nc.scalar.copy(ob, op_)
                    nc.sync.dma_start(out=out[e * 128:(e + 1) * 128, tsl], in_=ob)

    nc.compile()
    return nc


def _get_program():
    if "nc" not in _PROG:
        _PROG["nc"] = _build_program()
    return _PROG["nc"]


def kernel(x, wq, wk, wv, wo, rope_cos, rope_sin):
    from concourse.bass_utils import run_bass_kernel_spmd

    nc = _get_program()
    x = np.asarray(x, dtype=np.float32)
    wq = np.asarray(wq, dtype=np.float32)
    wk = np.asarray(wk, dtype=np.float32)
    wv = np.asarray(wv, dtype=np.float32)
    wo = np.asarray(wo, dtype=np.float32)
    cosT = np.asarray(rope_cos, dtype=np.float32).T  # [64, T]
    sinT = np.asarray(rope_sin, dtype=np.float32).T

    # even/odd -> [evens | odds] permutation of each head's rows of wq/wk
    perm = np.concatenate([np.arange(0, HD, 2), np.arange(1, HD, 2)])
    wq_p = wq.reshape(N_HEAD, HD, C)[:, perm, :]          # [16, 128, C]
    wk_p = wk.reshape(N_KV_HEAD, HD, C)[:, perm, :]       # [4, 128, C]
    wv_r = wv.reshape(N_KV_HEAD, HD, C)                   # [4, 128, C]

    cos2 = np.concatenate([cosT, cosT], axis=0).astype(np.float16)
    sin2 = np.concatenate([-sinT, sinT], axis=0).astype(np.float32)
    pswap = np.zeros((128, 128), dtype=np.float16)
    pswap[(np.arange(128) + 64) % 128, np.arange(128)] = 1.0
    negeye = (NEG * np.eye(128)).astype(np.float16)
    kt = np.arange(128)[:, None, None]
    r_ = np.arange(4)[None, :, None]
    qt = np.arange(512)[None, None, :]
    cmask = (qt < r_ * 128 + kt).astype(np.float16)       # [128, 4, 512]

    def part_major(a):  # [rows(c=n*128+p), m] -> [128(p), n, m]
        rows, m = a.shape
        return np.ascontiguousarray(
            a.reshape(rows // 128, 128, m).transpose(1, 0, 2))

    in_maps = []
    for core in range(N_CORES):
        b, g = core // 2, core % 2
        xT = x[b].T.astype(np.float16)                     # [C, T]
        wq_g = wq_p[8 * g:8 * g + 8]                       # [8, 128, C]
        wqH = np.stack([part_major(wq_g[hl].T.astype(np.float16))
                        for hl in range(8)])               # [8, 128, 16, 128]
        wkH = np.concatenate(
            [part_major(wk_p[2 * g + kv].T.astype(np.float16))
             for kv in range(2)], axis=2)                  # [128, 16, 256]
        wvH = np.concatenate(
            [part_major(wv_r[2 * g + kv].T.astype(np.float16))
             for kv in range(2)], axis=2)
        wo_g = wo[:, 1024 * g:1024 * (g + 1)]              # [C(e), 1024(hd)]
        woH = part_major(wo_g.T.astype(np.float16))
        # woH: rows = hd = hl*128 + p -> [128(p), 8(hl), C(e)]
        in_maps.append({
            "xH": part_major(xT),
            "wqH": wqH,
            "wkH": wkH,
            "wvH": wvH,
            "woH": woH,
            "cos2": cos2,
            "sin2": sin2,
            "pswap": pswap,
            "negeye": negeye,
            "cmask": cmask,
        })

    global _LAST_IN_MAPS
    _LAST_IN_MAPS = in_maps
    res = run_bass_kernel_spmd(nc, in_maps, list(range(N_CORES))).results
    outp = np.empty((B, T, C), dtype=np.float32)
    for b in range(B):
        outp[b] = (res[2 * b]["out"].astype(np.float32)
                   + res[2 * b + 1]["out"].astype(np.float32)).T
    return outp
